# revision 24
# baseline (speedup 1.0000x reference)
"""Trainium2 Bass kernel for nn_CompressedCausalAttention.

Sharding: 8 cores = 2 batches x 4 head-groups (2 heads each).
Per-core dataflow (chan-major "T" layouts are (channel partition, seq free)):
  host:    xpe = (x+pe)^T in bf16 (per batch), so the device never sees
           x/pe in f32 and does no adds (DMA 8MB -> 2MB per core).
  qkv:     qT,kT chan-major with bias applied on DVE (tensor_scalar_add,
           keeping ACT free for exps); v seq-major [t, j, h, 128] written
           directly by (xpe^T)-as-lhsT matmuls. Cols 0..63 of each head's
           128-wide v slot are ALL ONES: the AV matmul then emits the
           softmax denominator pre-broadcast 64-wide in rows 0..63, free.
           Only window 0's qkv runs up front; window i+1's q/k/v pieces
           are interleaved into window i's attention stream (q at window
           start, k/v only feed the diagonal blocks late in window i+1),
           so the qkv PE work overlaps the ACT-bound attention phase
           instead of serializing ahead of it.
  attn:    flash-style attention over (i=s-window, j=t-chunk) blocks,
           both heads' scores in one 2-bank PSUM tile, ONE merged exp per
           block on ACT (exp is the throughput co-bottleneck with PE),
           strict-causal staircase applied post-exp as a 0/1 triangle
           multiply on GpSimd (window 0) / DVE (windows 1-3), AV
           accumulation per head with the denominator riding along.
  norm:    1/den straight off AV rows 0..63 via reciprocal_approx_fast
           (DVE, psum in / sbuf out, partition offset 0 - the custom op
           mishandles offset inputs), then one DVE mul -> atn (bf16).
  phase 3: partial output projection outpT = Wc_mine^T-slice @ attnT.
Software pipelining: scores run 2 blocks ahead (PSUM sc-tag rotation
depth 2), AV lags 1 block, and window i's normalize+projection pieces
are spread one-per-block over the first 6 blocks of window i+1; the
final window's projections use the freed AV banks with per-head-split
contractions so they start after head0's normalize alone.
PSUM budget (8 banks): sc 2x2 + av0 2x1 + av1 2x1 = 8.
Host: shards inputs, sums the 4 per-batch partials, adds bc_eff
(v-bias folded through the output projection).
"""

import numpy as np
import ml_dtypes

S, B, C, H = 2048, 2, 512, 8
CC = C // H            # 64
HPC = 2                # heads per core
NCORE = 8
SW = 512               # s window (free dim of score tiles)
TCH = 128              # t chunk (partition dim of score tiles)
NW = S // SW           # 4 windows
TEMP = 1.0 / 8.0       # 1/sqrt(CC)
BIGNEG = -30000.0

_CACHE = {}


def _build_bass():
    import concourse.bass as bass
    import concourse.mybir as mybir
    import concourse.tile as tile
    from concourse import bacc

    f32 = mybir.dt.float32
    bf16 = mybir.dt.bfloat16
    fp8 = mybir.dt.float8e4
    DR = mybir.MatmulPerfMode.DoubleRow

    nc = bacc.Bacc("TRN2", target_bir_lowering=False)
    xt = nc.declare_dram_parameter("xt", [4, 128, S], fp8, isOutput=False)
    xtv = nc.declare_dram_parameter("xtv", [4, 128, S], bf16, isOutput=False)
    w3t = nc.declare_dram_parameter("w3t", [128, 4, 256], fp8, isOutput=False)
    w3v = nc.declare_dram_parameter("w3v", [128, 4, 128], bf16, isOutput=False)
    b3 = nc.declare_dram_parameter("b3", [128, 2], f32, isOutput=False)
    wct = nc.declare_dram_parameter("wct", [128, C], bf16, isOutput=False)
    tri = nc.declare_dram_parameter("tri", [128, 128], bf16, isOutput=False)
    outp = nc.declare_dram_parameter("outp", [C, S], bf16, isOutput=True)

    Ident = mybir.ActivationFunctionType.Identity
    Exp = mybir.ActivationFunctionType.Exp

    with tile.TileContext(nc) as tc:
        with (
            tc.tile_pool(name="singles", bufs=1) as singles,
            tc.tile_pool(name="pbp", bufs=4) as pbp,
            tc.tile_pool(name="atp", bufs=2) as atp,
            tc.tile_pool(name="rbp", bufs=2) as rbp,
            tc.tile_pool(name="osp", bufs=6) as osp,
            tc.tile_pool(name="ps", bufs=2, space="PSUM") as ps,
        ):
            # ---- inputs: the first-needed tensors (w3t chunk 0, window-0
            # xpe chunks) are triggered from the engine queues whose
            # preambles finish EARLIEST (GpSimd < Scalar < Vector < Tensor
            # < Sync), so transfers begin ~2.5us sooner than Sync-queue
            # issue would allow; everything else streams on Sync ----
            w3t_sb = singles.tile([128, 4, 256], fp8, tag="w3t")
            w3v_sb = singles.tile([128, 4, 128], bf16, tag="w3v")
            xpe = singles.tile([128, 4, S], fp8, tag="xpe")
            xpv = singles.tile([128, 4, S], bf16, tag="xpv")
            tri_sb = singles.tile([128, 128], bf16, tag="tri")
            b3_sb = singles.tile([128, 2], f32, tag="b3")
            nc.gpsimd.dma_start(out=w3t_sb[:, 0:2, :], in_=w3t[:, 0:2, :])
            nc.gpsimd.dma_start(out=xpe[:, 0, 0:SW], in_=xt[0, :, 0:SW])
            nc.gpsimd.dma_start(out=xpe[:, 2, 0:SW], in_=xt[2, :, 0:SW])
            nc.gpsimd.dma_start(out=tri_sb, in_=tri[:, :])
            nc.scalar.dma_start(out=w3t_sb[:, 2:4, :], in_=w3t[:, 2:4, :])
            nc.scalar.dma_start(out=xpe[:, 1, 0:SW], in_=xt[1, :, 0:SW])
            nc.scalar.dma_start(out=xpe[:, 3, 0:SW], in_=xt[3, :, 0:SW])
            nc.scalar.dma_start(out=b3_sb, in_=b3[:, :])
            nc.sync.dma_start(out=w3v_sb, in_=w3v[:, :, :])
            for k in range(4):
                nc.sync.dma_start(out=xpv[:, k, 0:SW], in_=xtv[k, :, 0:SW])
            for w in range(1, NW):
                sl = slice(w * SW, (w + 1) * SW)
                for k in range(4):
                    nc.sync.dma_start(out=xpe[:, k, sl], in_=xt[k, :, sl])
                for k in range(4):
                    nc.sync.dma_start(out=xpv[:, k, sl], in_=xtv[k, :, sl])
            wct_sb = singles.tile([128, C], bf16, tag="wct")
            nc.sync.dma_start(out=wct_sb, in_=wct[:, :])

            qT = singles.tile([128, S], bf16, tag="qT")
            kT = singles.tile([128, S], bf16, tag="kT")
            # v seq-major: [t(128), j(16), h(2), 128]; cols CC..127 of each
            # head slot are all ones, so AV rows CC..127 come out as the
            # softmax denominator already broadcast 64-wide (free on PE).
            vsb = singles.tile([128, 16, HPC, 128], bf16, tag="vsb")
            # memsets on GpSimd: keeps DVE free for the window-0
            # q/k bias-adds that gate the qkv pipeline
            warm = singles.tile([128, SW], bf16, tag="warm")
            nc.gpsimd.memset(warm, 0.0)
            nc.gpsimd.memset(vsb[:, :, :, 0:CC], 1.0)

            # p-state pre-warm: dependency-free dummy matmuls bridge the
            # gap from PE preamble end to the first input DMA landing
            # (~1-2us), keeping the clock ramp going; their garbage PSUM
            # output is overwritten by the first start=True real mm.
            for _ in range(3):
                wp = ps.tile([128, SW], f32, tag="sc", name="wp")
                nc.tensor.matmul(wp, lhsT=warm[:, 0:128], rhs=warm,
                                 start=True, stop=True)
            # ACT warm-up: force the Exp table load (~1.3us) early on a
            # tiny dummy, instead of lazily inside window 0's first exp
            wexp = singles.tile([1, 8], bf16, tag="wexp")
            nc.scalar.activation(out=wexp, in_=warm[0:1, 0:8], func=Exp,
                                 scale=1.0)

            # ---- qkv pieces (q/k bias on DVE, ACT reserved for exps).
            # Window 0's qkv runs pre-loop through dedicated sc-ring
            # tiles; every later window's q/k/v matmuls write into the
            # UNUSED [0:D] region of the current window's diagonal score
            # tiles (strict causality leaves 128/256/384 dead f32 columns
            # per head there), so they consume NO extra PSUM ring slots
            # and never perturb the depth-2 score/exp pipeline. ----
            # q/k/v matmuls run in fp8 DoubleRow: chunk PAIRS ride the
            # middle free dim of both operands (contraction 256/pass), so
            # each 512-deep projection is 2 accumulation steps at ~2x
            # throughput. Host pre-scales W by 32 (fp8 range) -- scores
            # come out x1024 (folded into the exp scale) and v x32
            # (folded into wct).
            def emit_q(w, blk, dst):
                sl = slice(w * SW, (w + 1) * SW)
                qp = ps.tile([128, SW], f32, tag="sc", name=f"qp{blk}_{w}")
                for k in (0, 2):
                    nc.tensor.matmul(
                        qp,
                        lhsT=w3t_sb[:, k:k + 2, blk * 128:(blk + 1) * 128],
                        rhs=xpe[:, k:k + 2, sl],
                        start=(k == 0), stop=(k == 2),
                        perf_mode=DR,
                    )
                nc.vector.tensor_scalar_add(
                    out=dst[:, sl], in0=qp,
                    scalar1=b3_sb[:, blk:blk + 1],
                )

            # v stays bf16: fp8 v costs ~3% relative error on the output
            # (the softmax-weighted mean shrinks signal and noise alike,
            # so v quantization error does NOT average down), which alone
            # would blow the 2e-2 budget
            def emit_v(w, half):
                vp = ps.tile([128, 2, HPC, CC], f32, tag="sc",
                             name=f"vp{half}_{w}")
                for tc_ in range(2):
                    t0 = (4 * w + 2 * half + tc_) * TCH
                    for k in range(4):
                        nc.tensor.matmul(
                            vp[:, tc_],
                            lhsT=xpv[:, k, t0:t0 + TCH],
                            rhs=w3v_sb[:, k, :],
                            start=(k == 0), stop=(k == 3),
                        )
                c0 = 4 * w + 2 * half
                nc.vector.tensor_copy(
                    out=vsb[:, c0:c0 + 2, :, CC:2 * CC], in_=vp,
                )

            # window-0 k is emitted in two column pieces through one tile
            # so sc(0,0) (which only needs kT[:,0:128]) can issue ~1us
            # earlier than a full 512-col k group would allow
            def emit_k0_split():
                kp = ps.tile([128, SW], f32, tag="sc", name="kp0")
                for k in (0, 2):
                    nc.tensor.matmul(
                        kp[:, 0:TCH],
                        lhsT=w3t_sb[:, k:k + 2, 128:256],
                        rhs=xpe[:, k:k + 2, 0:TCH],
                        start=(k == 0), stop=(k == 2),
                        perf_mode=DR, skip_group_check=True,
                    )
                nc.vector.tensor_scalar_add(
                    out=kT[:, 0:TCH], in0=kp[:, 0:TCH],
                    scalar1=b3_sb[:, 1:2],
                )
                emit_sc(0)
                for k in (0, 2):
                    nc.tensor.matmul(
                        kp[:, TCH:SW],
                        lhsT=w3t_sb[:, k:k + 2, 128:256],
                        rhs=xpe[:, k:k + 2, TCH:SW],
                        start=(k == 0), stop=(k == 2),
                        perf_mode=DR, skip_group_check=True,
                    )
                nc.vector.tensor_scalar_add(
                    out=kT[:, TCH:SW], in0=kp[:, TCH:SW],
                    scalar1=b3_sb[:, 1:2],
                )

            # ---- attention, flat software-pipelined loop. All sc-ring
            # piece pairs are emitted BEFORE that iteration's emit_sc so
            # the score stream always lands on quick-consumed piece slots
            # and keeps its depth-2 exp pipelining undisturbed. ----
            # per-window block order: j=0 (full, starts the AV accumulation),
            # then the short diagonal blocks (their exp->mask->AV latency is
            # hidden among long neighbors), then long off-diagonal blocks so
            # the window ends with deep PE work in flight
            blocks = []
            win_start = {}
            stop_j = {}
            for i in range(NW):
                js = list(range(4 * i + 4))
                win_start[i] = len(blocks)
                stop_j[i] = js[-1]
                blocks += [(i, j) for j in js]
            NB = len(blocks)
            sc_t = {}
            pb_t = {}
            av_t = {}

            def emit_sc(b):
                if b >= NB or b in sc_t:
                    return
                i, j = blocks[b]
                D = max(0, TCH * j - SW * i)
                if b < 2:
                    # fill phase: the av rings are still empty, so the
                    # first two blocks take per-head 1-bank tiles there --
                    # the sc ring then starts with ALL its slots holding
                    # quick-consumed qkv pieces and the whole fill runs
                    # without a single ring stall
                    ts_ = [
                        ps.tile([128, SW], f32, tag=f"av{h}",
                                name=f"sc{h}_{b}")
                        for h in range(HPC)
                    ]
                    for h in range(HPC):
                        nc.tensor.matmul(
                            ts_[h][:, D:SW],
                            lhsT=kT[h * CC:(h + 1) * CC,
                                    j * TCH:(j + 1) * TCH],
                            rhs=qT[h * CC:(h + 1) * CC,
                                   i * SW + D:(i + 1) * SW],
                            start=True, stop=True,
                        )
                    sc_t[b] = ts_
                    return
                t = ps.tile([128, HPC, SW], f32, tag="sc", name=f"sc_{b}")
                for h in range(HPC):
                    nc.tensor.matmul(
                        t[:, h, D:SW],
                        lhsT=kT[h * CC:(h + 1) * CC, j * TCH:(j + 1) * TCH],
                        rhs=qT[h * CC:(h + 1) * CC, i * SW + D:(i + 1) * SW],
                        start=True, stop=True,
                    )
                sc_t[b] = t

            # q,k are stored x32 (fp8 weight scaling), so scores are
            # x1024 -- folded into the exp scale
            TEMP_E = TEMP / 1024.0

            def emit_exp(b):
                i, j = blocks[b]
                D = max(0, TCH * j - SW * i)
                pb = pbp.tile([128, HPC, SW], bf16, tag="pb", name=f"pb_{b}")
                if b < 2:
                    for h in range(HPC):
                        nc.scalar.activation(
                            out=pb[:, h, D:SW], in_=sc_t[b][h][:, D:SW],
                            func=Exp, scale=TEMP_E)
                else:
                    nc.scalar.activation(out=pb[:, :, D:SW],
                                         in_=sc_t[b][:, :, D:SW],
                                         func=Exp, scale=TEMP_E)
                if j >= 4 * i:
                    # strict-causal staircase: zero the masked triangle
                    # post-exp. Window 0 runs it on GpSimd (DVE is busy
                    # with qkv biases/v copies); later windows use DVE,
                    # whose shorter launch latency keeps the lag-1 AV fed
                    # through the short diagonal-block cascade.
                    eng = nc.gpsimd if i == 0 else nc.vector
                    for h in range(HPC):
                        eng.tensor_mul(
                            out=pb[:, h, D:D + TCH], in0=pb[:, h, D:D + TCH],
                            in1=tri_sb,
                        )
                pb_t[b] = pb

            def emit_av(b):
                i, j = blocks[b]
                D = max(0, TCH * j - SW * i)
                if j == 0:
                    av_t[i] = [
                        ps.tile([128, SW], f32, tag=f"av{h}", name=f"av{h}_{i}")
                        for h in range(HPC)
                    ]
                for h in range(HPC):
                    nc.tensor.matmul(
                        av_t[i][h][:, D:SW], lhsT=vsb[:, j, h, :],
                        rhs=pb_t[b][:, h, D:SW],
                        start=(j == 0), stop=(j == stop_j[i]),
                    )
                del pb_t[b]

            # normalize + projection for window i, split into 4 pieces that
            # get interleaved into the next window's block stream
            atn_t = {}

            def emit_norm_h(i, h):
                # v cols 0..63 are all ones, so AV rows 0..63 come out as
                # the softmax denominator already broadcast 64-wide
                rcb = rbp.tile([CC, SW], f32, tag="rcb", name=f"rcb{h}_{i}")
                nc.vector.reciprocal_approx_fast(out=rcb, in_=av_t[i][h][0:CC, :])
                if h == 0:
                    atn_t[i] = atp.tile([128, SW], bf16, tag="atn", name=f"atn_{i}")
                nc.vector.tensor_mul(
                    out=atn_t[i][h * CC:(h + 1) * CC, :],
                    in0=av_t[i][h][CC:2 * CC, :], in1=rcb,
                )

            def emit_proj(i, d, ptag, on_act, split=False, pbufs=None):
                op = ps.tile([128, SW], f32, tag=ptag, bufs=pbufs,
                             name=f"op{d}_{i}")
                if split:
                    # per-head contract halves: the first matmul needs only
                    # head0's normalize, shortening the end-of-kernel chain
                    for h in range(HPC):
                        nc.tensor.matmul(
                            op, lhsT=wct_sb[h * CC:(h + 1) * CC,
                                           d * 128:(d + 1) * 128],
                            rhs=atn_t[i][h * CC:(h + 1) * CC, :],
                            start=(h == 0), stop=(h == 1),
                        )
                else:
                    nc.tensor.matmul(
                        op, lhsT=wct_sb[:, d * 128:(d + 1) * 128],
                        rhs=atn_t[i], start=True, stop=True,
                    )
                ob = osp.tile([128, SW], bf16, tag="ob", name=f"ob{d}_{i}")
                if on_act:
                    nc.scalar.copy(out=ob, in_=op)
                else:
                    nc.vector.tensor_copy(out=ob, in_=op)
                nc.sync.dma_start(
                    out=outp[d * 128:(d + 1) * 128, i * SW:(i + 1) * SW], in_=ob,
                )

            # Piece schedule. Window 0's q + split-k precede the stream
            # (v(0) halves ride pos 0-1); per window i:
            #   pos 0:   norm_h0(i-1) right after window i-1's last AV
            #   pos 1:   norm_h1(i-1)
            #   pos 3/4: proj(i-1, 0/1) through the freed av0/av1 slots
            #   pos 5:   proj(i-2, 2/3) sc pair (deferred one window so
            #            the PE-dense window 1 never hosts them)
            #   mid:     [q,k](i+1) pair, then [v,v](i+1) pair two blocks
            #            later -- mid-window, where ACT is the binding
            #            engine and PE slack absorbs the ring-slot waits
            qk_pos = {0: 1, 1: 4, 2: 6}       # window -> pos of [q,k](w+1)
            vv_pos = {0: 3, 1: 6, 2: 8}       # window -> pos of [v,v](w+1)
            emit_q(0, 0, qT)
            emit_k0_split()
            emit_sc(1)
            for b in range(NB):
                i, j = blocks[b]
                pos = b - win_start[i]
                # ready AV matmuls lead the PE FIFO so piece stalls never
                # block them
                if b > 0:
                    emit_av(b - 1)
                    pi, pj = blocks[b - 1]
                    if pj == stop_j[pi]:
                        emit_norm_h(pi, 0)
                if i >= 2 and pos == 5:
                    emit_proj(i - 2, 2, "sc", on_act=False)
                    emit_proj(i - 2, 3, "sc", on_act=False)
                if i == 0 and pos == 0:
                    emit_v(0, 0)
                    emit_v(0, 1)
                if qk_pos.get(i) == pos:
                    emit_q(i + 1, 0, qT)
                    emit_q(i + 1, 1, kT)
                if vv_pos.get(i) == pos:
                    emit_v(i + 1, 0)
                    emit_v(i + 1, 1)
                emit_sc(b + 2)
                emit_exp(b)
                if i > 0:
                    if pos == 1:
                        emit_norm_h(i - 1, 1)
                    elif pos == 3:
                        emit_proj(i - 1, 0, "av0", on_act=False)
                    elif pos == 4:
                        emit_proj(i - 1, 1, "av1", on_act=False)
            emit_av(NB - 1)
            # tail: proj(NW-2, 2/3) have no norm dependency -- they run
            # immediately on the freed sc slots, keeping the PE hot while
            # the DVE norm chain (recip+mul x2 heads) for the last window
            # drains; then the final 4 projs (all h0 contractions first,
            # so no head-1 stall blocks a head-0-ready matmul), casts
            # split ACT/DVE, eager DMA per 128-row quarter.
            emit_proj(NW - 2, 2, "sc", on_act=True)
            emit_proj(NW - 2, 3, "sc", on_act=True)
            emit_norm_h(NW - 1, 0)
            emit_norm_h(NW - 1, 1)
            ftag = {0: "av0", 1: "sc", 2: "av0", 3: "sc"}
            fop = [
                ps.tile([128, SW], f32, tag=ftag[d], name=f"fop{d}")
                for d in range(4)
            ]
            for h in range(HPC):
                for d in range(4):
                    nc.tensor.matmul(
                        fop[d],
                        lhsT=wct_sb[h * CC:(h + 1) * CC,
                                    d * 128:(d + 1) * 128],
                        rhs=atn_t[NW - 1][h * CC:(h + 1) * CC, :],
                        start=(h == 0), stop=(h == 1),
                        skip_group_check=True,
                    )
            for d in range(4):
                fob = osp.tile([128, SW], bf16, tag="ob", name=f"fob{d}")
                if d < 2:
                    nc.scalar.copy(out=fob, in_=fop[d])
                else:
                    nc.vector.tensor_copy(out=fob, in_=fop[d])
                nc.sync.dma_start(
                    out=outp[d * 128:(d + 1) * 128,
                             (NW - 1) * SW:NW * SW],
                    in_=fob,
                )

    nc.compile()
    return nc


def _get_nc():
    if "nc" not in _CACHE:
        _CACHE["nc"] = _build_bass()
    return _CACHE["nc"]


WSCALE = 32.0   # fp8 range scaling for Wqkv (~N(0,0.02) would be subnormal)


def _make_in_maps(x, pe, Wqkv, bqkv, Wc):
    bf = ml_dtypes.bfloat16
    f8 = ml_dtypes.float8_e4m3   # TRN FP8_EXP4-compatible (max +-240)
    tt = np.arange(128)[:, None]   # t (pb partition)
    kk = np.arange(128)[None, :]   # s_local - D (pb free col)
    # keep-mask: pb[t, c] survives iff c >= t (strictly-causal staircase)
    tri = (kk >= tt).astype(np.float32).astype(bf)

    xt_b = {}
    xtv_b = {}
    for b in range(B):
        t = (x[:, b, :] + pe[:, b, :]).T     # [C, S]
        t8 = np.clip(t, -240.0, 240.0).astype(f8)
        xt_b[b] = np.ascontiguousarray(t8.reshape(4, 128, S))
        xtv_b[b] = np.ascontiguousarray(t.astype(bf).reshape(4, 128, S))

    in_maps = []
    for core in range(NCORE):
        b, hg = core // 4, core % 4
        lo = hg * 128
        W3 = np.concatenate(
            [Wqkv[lo:lo + 128], Wqkv[C + lo:C + lo + 128],
             Wqkv[2 * C + lo:2 * C + lo + 128]])
        w3full = W3.T.reshape(4, 128, 384).transpose(1, 0, 2)  # [128,4,384]
        w3t = np.clip(w3full[:, :, 0:256] * WSCALE, -240.0, 240.0)
        w3t = np.ascontiguousarray(w3t).astype(f8)
        w3v = np.ascontiguousarray(w3full[:, :, 256:384]).astype(bf)
        b3 = np.stack([bqkv[lo:lo + 128], bqkv[C + lo:C + lo + 128]], axis=1)
        b3 = np.ascontiguousarray(b3).astype(np.float32) * np.float32(WSCALE)
        wct = np.ascontiguousarray(Wc[:, lo:lo + 128].T).astype(bf)
        in_maps.append({
            "xt": xt_b[b], "xtv": xtv_b[b], "w3t": w3t, "w3v": w3v,
            "b3": b3, "wct": wct, "tri": tri,
        })
    return in_maps


def _numpy_fallback(x, pe, content_mask, Wqkv, bqkv, Wc, bc):
    xpe = (x + pe).astype(np.float32)
    qkv = xpe.reshape(-1, C) @ Wqkv.T + bqkv
    qkv = qkv.reshape(S, B, 3 * C)
    q, k, v = np.split(qkv, 3, axis=-1)
    q = q.reshape(S, B, H, CC)
    k = k.reshape(S, B, H, CC)
    v = v.reshape(S, B, H, CC)
    out = np.empty((S, B, C), np.float32)
    for b in range(B):
        for h in range(H):
            sc = (q[:, b, h] @ k[:, b, h].T) * np.float32(TEMP)
            sc = np.where(content_mask[:, :, b], -np.inf, sc)
            sc = sc - sc.max(axis=1, keepdims=True)
            p = np.exp(sc)
            p /= p.sum(axis=1, keepdims=True)
            out[:, b, h * CC:(h + 1) * CC] = p @ v[:, b, h]
    return (out.reshape(-1, C) @ Wc.T + bc).reshape(S, B, C).astype(np.float32)


def kernel(x, pe, content_mask, pad, Wqkv, bqkv, Wc, bc):
    x = np.asarray(x, dtype=np.float32)
    pe = np.asarray(pe, dtype=np.float32)
    content_mask = np.asarray(content_mask)
    Wqkv = np.asarray(Wqkv, dtype=np.float32)
    bqkv = np.asarray(bqkv, dtype=np.float32)
    Wc = np.asarray(Wc, dtype=np.float32)
    bc = np.asarray(bc, dtype=np.float32)

    idx = np.arange(S)
    causal = idx[None, :] > idx[:, None]
    if not np.array_equal(content_mask, np.broadcast_to(causal[:, :, None], (S, S, B))):
        return _numpy_fallback(x, pe, content_mask, Wqkv, bqkv, Wc, bc)

    from concourse.bass_utils import run_bass_kernel_spmd

    nc = _get_nc()
    in_maps = _make_in_maps(x, pe, Wqkv, bqkv, Wc)
    res = run_bass_kernel_spmd(nc, in_maps, core_ids=list(range(NCORE)))
    out = np.empty((S, B, C), np.float32)
    bc_eff = bc + Wc @ bqkv[2 * C:3 * C]   # v-bias folded through the output proj
    for b in range(B):
        acc = res.results[b * 4]["outp"].astype(np.float32).copy()
        for g in range(1, 4):
            acc += res.results[b * 4 + g]["outp"]
        out[:, b, :] = acc.T + bc_eff
    return out



# revision 25
# speedup vs baseline: 1.1865x; 1.1865x over previous
"""Trainium2 Bass kernel for nn_CompressedCausalAttention.

Sharding: 8 cores = 2 batches x 4 head-groups (2 heads each).
Per-core dataflow (chan-major "T" layouts are (channel partition, seq free)):
  host:    xpe = (x+pe)^T in bf16 (per batch), so the device never sees
           x/pe in f32 and does no adds (DMA 8MB -> 2MB per core).
  qkv:     qT,kT chan-major with bias applied on DVE (tensor_scalar_add,
           keeping ACT free for exps); v seq-major [t, j, h, 128] written
           directly by (xpe^T)-as-lhsT matmuls. Cols 0..63 of each head's
           128-wide v slot are ALL ONES: the AV matmul then emits the
           softmax denominator pre-broadcast 64-wide in rows 0..63, free.
           Only window 0's qkv runs up front; window i+1's q/k/v pieces
           are interleaved into window i's attention stream (q at window
           start, k/v only feed the diagonal blocks late in window i+1),
           so the qkv PE work overlaps the ACT-bound attention phase
           instead of serializing ahead of it.
  attn:    flash-style attention over (i=s-window, j=t-chunk) blocks,
           both heads' scores in one 2-bank PSUM tile, ONE merged exp per
           block on ACT (exp is the throughput co-bottleneck with PE),
           strict-causal staircase applied post-exp as a 0/1 triangle
           multiply on GpSimd (window 0) / DVE (windows 1-3), AV
           accumulation per head with the denominator riding along.
  norm:    1/den straight off AV rows 0..63 via reciprocal_approx_fast
           (DVE, psum in / sbuf out, partition offset 0 - the custom op
           mishandles offset inputs), then one DVE mul -> atn (bf16).
  phase 3: partial output projection outpT = Wc_mine^T-slice @ attnT.
Software pipelining: scores run 2 blocks ahead (PSUM sc-tag rotation
depth 2), AV lags 1 block, and window i's normalize+projection pieces
are spread one-per-block over the first 6 blocks of window i+1; the
final window's projections use the freed AV banks with per-head-split
contractions so they start after head0's normalize alone.
PSUM budget (8 banks): sc 2x2 + av0 2x1 + av1 2x1 = 8.
Host: shards inputs, sums the 4 per-batch partials, adds bc_eff
(v-bias folded through the output projection).
"""

import numpy as np
import ml_dtypes

S, B, C, H = 2048, 2, 512, 8
CC = C // H            # 64
HPC = 2                # heads per core
NCORE = 8
SW = 512               # s window (free dim of score tiles)
TCH = 128              # t chunk (partition dim of score tiles)
NW = S // SW           # 4 windows
TEMP = 1.0 / 8.0       # 1/sqrt(CC)
BIGNEG = -30000.0

_CACHE = {}


def _build_bass():
    import concourse.bass as bass
    import concourse.mybir as mybir
    import concourse.tile as tile
    from concourse import bacc

    f32 = mybir.dt.float32
    bf16 = mybir.dt.bfloat16
    fp8 = mybir.dt.float8e4
    DR = mybir.MatmulPerfMode.DoubleRow

    nc = bacc.Bacc("TRN2", target_bir_lowering=False)
    xt = nc.declare_dram_parameter("xt", [4, 128, S], fp8, isOutput=False)
    xtv = nc.declare_dram_parameter("xtv", [4, 128, S], bf16, isOutput=False)
    w3t = nc.declare_dram_parameter("w3t", [128, 4, 256], fp8, isOutput=False)
    w3v = nc.declare_dram_parameter("w3v", [128, 4, 128], bf16, isOutput=False)
    b3 = nc.declare_dram_parameter("b3", [128, 2], f32, isOutput=False)
    wct = nc.declare_dram_parameter("wct", [128, C], bf16, isOutput=False)
    tri = nc.declare_dram_parameter("tri", [128, 128], bf16, isOutput=False)
    outp = nc.declare_dram_parameter("outp", [C, S], bf16, isOutput=True)

    Ident = mybir.ActivationFunctionType.Identity
    Exp = mybir.ActivationFunctionType.Exp

    with tile.TileContext(nc) as tc:
        with (
            tc.tile_pool(name="singles", bufs=1) as singles,
            tc.tile_pool(name="pbp", bufs=4) as pbp,
            tc.tile_pool(name="atp", bufs=2) as atp,
            tc.tile_pool(name="rbp", bufs=2) as rbp,
            tc.tile_pool(name="osp", bufs=6) as osp,
            tc.tile_pool(name="ps", bufs=2, space="PSUM") as ps,
        ):
            # ---- inputs: the first-needed tensors (w3t chunk 0, window-0
            # xpe chunks) are triggered from the engine queues whose
            # preambles finish EARLIEST (GpSimd < Scalar < Vector < Tensor
            # < Sync), so transfers begin ~2.5us sooner than Sync-queue
            # issue would allow; everything else streams on Sync ----
            w3t_sb = singles.tile([128, 4, 256], fp8, tag="w3t")
            w3v_sb = singles.tile([128, 4, 128], bf16, tag="w3v")
            xpe = singles.tile([128, 4, S], fp8, tag="xpe")
            xpv = singles.tile([128, 4, S], bf16, tag="xpv")
            tri_sb = singles.tile([128, 128], bf16, tag="tri")
            b3_sb = singles.tile([128, 2], f32, tag="b3")
            nc.gpsimd.dma_start(out=w3t_sb[:, 0:2, :], in_=w3t[:, 0:2, :])
            nc.gpsimd.dma_start(out=xpe[:, 0, 0:SW], in_=xt[0, :, 0:SW])
            nc.gpsimd.dma_start(out=xpe[:, 2, 0:SW], in_=xt[2, :, 0:SW])
            nc.gpsimd.dma_start(out=tri_sb, in_=tri[:, :])
            nc.scalar.dma_start(out=w3t_sb[:, 2:4, :], in_=w3t[:, 2:4, :])
            nc.scalar.dma_start(out=xpe[:, 1, 0:SW], in_=xt[1, :, 0:SW])
            nc.scalar.dma_start(out=xpe[:, 3, 0:SW], in_=xt[3, :, 0:SW])
            nc.scalar.dma_start(out=b3_sb, in_=b3[:, :])
            nc.sync.dma_start(out=w3v_sb, in_=w3v[:, :, :])
            for k in range(4):
                nc.sync.dma_start(out=xpv[:, k, 0:SW], in_=xtv[k, :, 0:SW])
            for w in range(1, NW):
                sl = slice(w * SW, (w + 1) * SW)
                for k in range(4):
                    nc.sync.dma_start(out=xpe[:, k, sl], in_=xt[k, :, sl])
                for k in range(4):
                    nc.sync.dma_start(out=xpv[:, k, sl], in_=xtv[k, :, sl])
            wct_sb = singles.tile([128, C], bf16, tag="wct")
            nc.sync.dma_start(out=wct_sb, in_=wct[:, :])

            qT = singles.tile([128, S], bf16, tag="qT")
            kT = singles.tile([128, S], bf16, tag="kT")
            # v seq-major: [t(128), j(16), h(2), 128]; cols CC..127 of each
            # head slot are all ones, so AV rows CC..127 come out as the
            # softmax denominator already broadcast 64-wide (free on PE).
            vsb = singles.tile([128, 16, HPC, 128], bf16, tag="vsb")
            # memsets on GpSimd: keeps DVE free for the window-0
            # q/k bias-adds that gate the qkv pipeline
            warm = singles.tile([128, SW], bf16, tag="warm")
            nc.gpsimd.memset(warm, 0.0)
            nc.gpsimd.memset(vsb[:, :, :, 0:CC], 1.0)

            # p-state pre-warm: dependency-free dummy matmuls bridge the
            # gap from PE preamble end to the first input DMA landing
            # (~1-2us), keeping the clock ramp going; their garbage PSUM
            # output is overwritten by the first start=True real mm.
            for _ in range(3):
                wp = ps.tile([128, SW], f32, tag="sc", name="wp")
                nc.tensor.matmul(wp, lhsT=warm[:, 0:128], rhs=warm,
                                 start=True, stop=True)
            # ACT warm-up: force the Exp table load (~1.3us) early on a
            # tiny dummy, instead of lazily inside window 0's first exp
            wexp = singles.tile([1, 8], bf16, tag="wexp")
            nc.scalar.activation(out=wexp, in_=warm[0:1, 0:8], func=Exp,
                                 scale=1.0)

            # ---- qkv pieces (q/k bias on DVE, ACT reserved for exps).
            # Window 0's qkv runs pre-loop through dedicated sc-ring
            # tiles; every later window's q/k/v matmuls write into the
            # UNUSED [0:D] region of the current window's diagonal score
            # tiles (strict causality leaves 128/256/384 dead f32 columns
            # per head there), so they consume NO extra PSUM ring slots
            # and never perturb the depth-2 score/exp pipeline. ----
            # q/k/v matmuls run in fp8 DoubleRow: chunk PAIRS ride the
            # middle free dim of both operands (contraction 256/pass), so
            # each 512-deep projection is 2 accumulation steps at ~2x
            # throughput. Host pre-scales W by 32 (fp8 range) -- scores
            # come out x1024 (folded into the exp scale) and v x32
            # (folded into wct).
            def emit_q(w, blk, dst):
                sl = slice(w * SW, (w + 1) * SW)
                qp = ps.tile([128, SW], f32, tag="sc", name=f"qp{blk}_{w}")
                for k in (0, 2):
                    nc.tensor.matmul(
                        qp,
                        lhsT=w3t_sb[:, k:k + 2, blk * 128:(blk + 1) * 128],
                        rhs=xpe[:, k:k + 2, sl],
                        start=(k == 0), stop=(k == 2),
                        perf_mode=DR,
                    )
                nc.vector.tensor_scalar_add(
                    out=dst[:, sl], in0=qp,
                    scalar1=b3_sb[:, blk:blk + 1],
                )

            # v stays bf16: fp8 v costs ~3% relative error on the output
            # (the softmax-weighted mean shrinks signal and noise alike,
            # so v quantization error does NOT average down), which alone
            # would blow the 2e-2 budget
            def emit_v(w, half):
                vp = ps.tile([128, 2, HPC, CC], f32, tag="sc",
                             name=f"vp{half}_{w}")
                for tc_ in range(2):
                    t0 = (4 * w + 2 * half + tc_) * TCH
                    for k in range(4):
                        nc.tensor.matmul(
                            vp[:, tc_],
                            lhsT=xpv[:, k, t0:t0 + TCH],
                            rhs=w3v_sb[:, k, :],
                            start=(k == 0), stop=(k == 3),
                        )
                c0 = 4 * w + 2 * half
                nc.vector.tensor_copy(
                    out=vsb[:, c0:c0 + 2, :, CC:2 * CC], in_=vp,
                )

            # window-0 k is emitted in two column pieces through one tile
            # so sc(0,0) (which only needs kT[:,0:128]) can issue ~1us
            # earlier than a full 512-col k group would allow
            def emit_k0_split():
                kp = ps.tile([128, SW], f32, tag="sc", name="kp0")
                for k in (0, 2):
                    nc.tensor.matmul(
                        kp[:, 0:TCH],
                        lhsT=w3t_sb[:, k:k + 2, 128:256],
                        rhs=xpe[:, k:k + 2, 0:TCH],
                        start=(k == 0), stop=(k == 2),
                        perf_mode=DR, skip_group_check=True,
                    )
                nc.vector.tensor_scalar_add(
                    out=kT[:, 0:TCH], in0=kp[:, 0:TCH],
                    scalar1=b3_sb[:, 1:2],
                )
                emit_sc(0)
                for k in (0, 2):
                    nc.tensor.matmul(
                        kp[:, TCH:SW],
                        lhsT=w3t_sb[:, k:k + 2, 128:256],
                        rhs=xpe[:, k:k + 2, TCH:SW],
                        start=(k == 0), stop=(k == 2),
                        perf_mode=DR, skip_group_check=True,
                    )
                nc.vector.tensor_scalar_add(
                    out=kT[:, TCH:SW], in0=kp[:, TCH:SW],
                    scalar1=b3_sb[:, 1:2],
                )

            # ---- attention, flat software-pipelined loop. All sc-ring
            # piece pairs are emitted BEFORE that iteration's emit_sc so
            # the score stream always lands on quick-consumed piece slots
            # and keeps its depth-2 exp pipelining undisturbed. ----
            # per-window block order: j=0 (full, starts the AV accumulation),
            # then the short diagonal blocks (their exp->mask->AV latency is
            # hidden among long neighbors), then long off-diagonal blocks so
            # the window ends with deep PE work in flight
            blocks = []
            win_start = {}
            stop_j = {}
            for i in range(NW):
                js = list(range(4 * i + 4))
                win_start[i] = len(blocks)
                stop_j[i] = js[-1]
                blocks += [(i, j) for j in js]
            NB = len(blocks)
            sc_t = {}
            pb_t = {}
            av_t = {}

            def emit_sc(b):
                if b >= NB or b in sc_t:
                    return
                i, j = blocks[b]
                D = max(0, TCH * j - SW * i)
                if b < 2:
                    # fill phase: the av rings are still empty, so the
                    # first two blocks take per-head 1-bank tiles there --
                    # the sc ring then starts with ALL its slots holding
                    # quick-consumed qkv pieces and the whole fill runs
                    # without a single ring stall
                    ts_ = [
                        ps.tile([128, SW], f32, tag=f"av{h}",
                                name=f"sc{h}_{b}")
                        for h in range(HPC)
                    ]
                    for h in range(HPC):
                        nc.tensor.matmul(
                            ts_[h][:, D:SW],
                            lhsT=kT[h * CC:(h + 1) * CC,
                                    j * TCH:(j + 1) * TCH],
                            rhs=qT[h * CC:(h + 1) * CC,
                                   i * SW + D:(i + 1) * SW],
                            start=True, stop=True,
                        )
                    sc_t[b] = ts_
                    return
                t = ps.tile([128, HPC, SW], f32, tag="sc", name=f"sc_{b}")
                for h in range(HPC):
                    nc.tensor.matmul(
                        t[:, h, D:SW],
                        lhsT=kT[h * CC:(h + 1) * CC, j * TCH:(j + 1) * TCH],
                        rhs=qT[h * CC:(h + 1) * CC, i * SW + D:(i + 1) * SW],
                        start=True, stop=True,
                    )
                sc_t[b] = t

            # q,k are stored x32 (fp8 weight scaling), so scores are
            # x1024 -- folded into the exp scale
            TEMP_E = TEMP / 1024.0

            def emit_exp(b):
                i, j = blocks[b]
                D = max(0, TCH * j - SW * i)
                pb = pbp.tile([128, HPC, SW], bf16, tag="pb", name=f"pb_{b}")
                if b < 2:
                    for h in range(HPC):
                        nc.scalar.activation(
                            out=pb[:, h, D:SW], in_=sc_t[b][h][:, D:SW],
                            func=Exp, scale=TEMP_E)
                else:
                    nc.scalar.activation(out=pb[:, :, D:SW],
                                         in_=sc_t[b][:, :, D:SW],
                                         func=Exp, scale=TEMP_E)
                if j >= 4 * i:
                    # strict-causal staircase: zero the masked triangle
                    # post-exp. Window 0 runs it on GpSimd (DVE is busy
                    # with qkv biases/v copies); later windows use DVE,
                    # whose shorter launch latency keeps the lag-1 AV fed
                    # through the short diagonal-block cascade.
                    eng = nc.gpsimd if i == 0 else nc.vector
                    for h in range(HPC):
                        eng.tensor_mul(
                            out=pb[:, h, D:D + TCH], in0=pb[:, h, D:D + TCH],
                            in1=tri_sb,
                        )
                pb_t[b] = pb

            def emit_av(b):
                i, j = blocks[b]
                D = max(0, TCH * j - SW * i)
                if j == 0:
                    av_t[i] = [
                        ps.tile([128, SW], f32, tag=f"av{h}", name=f"av{h}_{i}")
                        for h in range(HPC)
                    ]
                for h in range(HPC):
                    nc.tensor.matmul(
                        av_t[i][h][:, D:SW], lhsT=vsb[:, j, h, :],
                        rhs=pb_t[b][:, h, D:SW],
                        start=(j == 0), stop=(j == stop_j[i]),
                    )
                del pb_t[b]

            # normalize + projection for window i, split into 4 pieces that
            # get interleaved into the next window's block stream
            atn_t = {}

            def emit_norm_h(i, h):
                # v cols 0..63 are all ones, so AV rows 0..63 come out as
                # the softmax denominator already broadcast 64-wide
                rcb = rbp.tile([CC, SW], f32, tag="rcb", name=f"rcb{h}_{i}")
                nc.vector.reciprocal_approx_fast(out=rcb, in_=av_t[i][h][0:CC, :])
                if h == 0:
                    atn_t[i] = atp.tile([128, SW], bf16, tag="atn", name=f"atn_{i}")
                nc.vector.tensor_mul(
                    out=atn_t[i][h * CC:(h + 1) * CC, :],
                    in0=av_t[i][h][CC:2 * CC, :], in1=rcb,
                )

            def emit_proj(i, d, ptag, on_act, split=False, pbufs=None):
                op = ps.tile([128, SW], f32, tag=ptag, bufs=pbufs,
                             name=f"op{d}_{i}")
                if split:
                    # per-head contract halves: the first matmul needs only
                    # head0's normalize, shortening the end-of-kernel chain
                    for h in range(HPC):
                        nc.tensor.matmul(
                            op, lhsT=wct_sb[h * CC:(h + 1) * CC,
                                           d * 128:(d + 1) * 128],
                            rhs=atn_t[i][h * CC:(h + 1) * CC, :],
                            start=(h == 0), stop=(h == 1),
                        )
                else:
                    nc.tensor.matmul(
                        op, lhsT=wct_sb[:, d * 128:(d + 1) * 128],
                        rhs=atn_t[i], start=True, stop=True,
                    )
                ob = osp.tile([128, SW], bf16, tag="ob", name=f"ob{d}_{i}")
                if on_act:
                    nc.scalar.copy(out=ob, in_=op)
                else:
                    nc.vector.tensor_copy(out=ob, in_=op)
                nc.sync.dma_start(
                    out=outp[d * 128:(d + 1) * 128, i * SW:(i + 1) * SW], in_=ob,
                )

            # Piece schedule. Window 0's q + split-k precede the stream
            # (v(0) halves ride pos 0-1); per window i:
            #   pos 0:   norm_h0(i-1) right after window i-1's last AV
            #   pos 1:   norm_h1(i-1)
            #   pos 3/4: proj(i-1, 0/1) through the freed av0/av1 slots
            #   pos 5:   proj(i-2, 2/3) sc pair (deferred one window so
            #            the PE-dense window 1 never hosts them)
            #   mid:     [q,k](i+1) pair, then [v,v](i+1) pair two blocks
            #            later -- mid-window, where ACT is the binding
            #            engine and PE slack absorbs the ring-slot waits
            qk_pos = {0: 1, 1: 4, 2: 6}       # window -> pos of [q,k](w+1)
            vv_pos = {0: 3, 1: 6, 2: 8}       # window -> pos of [v,v](w+1)
            emit_q(0, 0, qT)
            emit_k0_split()
            emit_sc(1)
            for b in range(NB):
                i, j = blocks[b]
                pos = b - win_start[i]
                # ready AV matmuls lead the PE FIFO so piece stalls never
                # block them
                if b > 0:
                    emit_av(b - 1)
                    pi, pj = blocks[b - 1]
                    if pj == stop_j[pi]:
                        emit_norm_h(pi, 0)
                if i >= 2 and pos == 5:
                    emit_proj(i - 2, 2, "sc", on_act=False)
                    emit_proj(i - 2, 3, "sc", on_act=False)
                if i == 0 and pos == 0:
                    emit_v(0, 0)
                    emit_v(0, 1)
                if qk_pos.get(i) == pos:
                    emit_q(i + 1, 0, qT)
                    emit_q(i + 1, 1, kT)
                if vv_pos.get(i) == pos:
                    emit_v(i + 1, 0)
                    emit_v(i + 1, 1)
                emit_sc(b + 2)
                emit_exp(b)
                if i > 0:
                    if pos == 1:
                        emit_norm_h(i - 1, 1)
                    elif pos == 3:
                        emit_proj(i - 1, 0, "av0", on_act=False)
                    elif pos == 4:
                        emit_proj(i - 1, 1, "av1", on_act=False)
            emit_av(NB - 1)
            # tail: proj(NW-2, 2/3) have no norm dependency -- they run
            # immediately on the freed sc slots, keeping the PE hot while
            # the DVE norm chain (recip+mul x2 heads) for the last window
            # drains; then the final 4 projs (all h0 contractions first,
            # so no head-1 stall blocks a head-0-ready matmul), casts
            # split ACT/DVE, eager DMA per 128-row quarter.
            emit_proj(NW - 2, 2, "sc", on_act=True)
            emit_proj(NW - 2, 3, "sc", on_act=True)
            # final-window normalize, chunked by s-halves so the first
            # projection casts fire ~0.7us earlier than a full-width norm
            # chain would allow (recips stay full-width: the custom DVE op
            # mishandles offset inputs)
            fi = NW - 1
            rc = []
            for h in range(HPC):
                r = rbp.tile([CC, SW], f32, tag="rcb", name=f"frc{h}")
                rc.append(r)
            atn3 = atp.tile([128, SW], bf16, tag="atn", name="atn_f")
            atn_t[fi] = atn3
            nc.vector.reciprocal_approx_fast(out=rc[0], in_=av_t[fi][0][0:CC, :])
            for half in range(2):
                s0 = half * 256
                nc.vector.tensor_mul(
                    out=atn3[0:CC, s0:s0 + 256],
                    in0=av_t[fi][0][CC:2 * CC, s0:s0 + 256],
                    in1=rc[0][:, s0:s0 + 256],
                )
            nc.vector.reciprocal_approx_fast(out=rc[1], in_=av_t[fi][1][0:CC, :])
            for half in range(2):
                s0 = half * 256
                nc.vector.tensor_mul(
                    out=atn3[CC:2 * CC, s0:s0 + 256],
                    in0=av_t[fi][1][CC:2 * CC, s0:s0 + 256],
                    in1=rc[1][:, s0:s0 + 256],
                )
            ftag = {0: "av0", 1: "sc", 2: "av0", 3: "sc"}
            for d in range(4):
                fp_ = ps.tile([128, SW], f32, tag=ftag[d], name=f"fop{d}")
                for h in range(HPC):
                    for half in range(2):
                        s0 = half * 256
                        nc.tensor.matmul(
                            fp_[:, s0:s0 + 256],
                            lhsT=wct_sb[h * CC:(h + 1) * CC,
                                        d * 128:(d + 1) * 128],
                            rhs=atn3[h * CC:(h + 1) * CC, s0:s0 + 256],
                            start=(h == 0), stop=(h == 1),
                            skip_group_check=True,
                        )
                fob = osp.tile([128, SW], bf16, tag="ob", name=f"fob{d}")
                for half in range(2):
                    s0 = half * 256
                    if d < 2:
                        nc.scalar.copy(out=fob[:, s0:s0 + 256],
                                       in_=fp_[:, s0:s0 + 256])
                    else:
                        nc.vector.tensor_copy(out=fob[:, s0:s0 + 256],
                                              in_=fp_[:, s0:s0 + 256])
                    eng = nc.sync if (d + half) % 2 == 0 else nc.gpsimd
                    eng.dma_start(
                        out=outp[d * 128:(d + 1) * 128,
                                 fi * SW + s0:fi * SW + s0 + 256],
                        in_=fob[:, s0:s0 + 256],
                    )

    nc.compile()
    return nc


def _get_nc():
    if "nc" not in _CACHE:
        _CACHE["nc"] = _build_bass()
    return _CACHE["nc"]


WSCALE = 32.0   # fp8 range scaling for Wqkv (~N(0,0.02) would be subnormal)


def _make_in_maps(x, pe, Wqkv, bqkv, Wc):
    bf = ml_dtypes.bfloat16
    f8 = ml_dtypes.float8_e4m3   # TRN FP8_EXP4-compatible (max +-240)
    tt = np.arange(128)[:, None]   # t (pb partition)
    kk = np.arange(128)[None, :]   # s_local - D (pb free col)
    # keep-mask: pb[t, c] survives iff c >= t (strictly-causal staircase)
    tri = (kk >= tt).astype(np.float32).astype(bf)

    xt_b = {}
    xtv_b = {}
    for b in range(B):
        t = (x[:, b, :] + pe[:, b, :]).T     # [C, S]
        t8 = np.clip(t, -240.0, 240.0).astype(f8)
        xt_b[b] = np.ascontiguousarray(t8.reshape(4, 128, S))
        xtv_b[b] = np.ascontiguousarray(t.astype(bf).reshape(4, 128, S))

    in_maps = []
    for core in range(NCORE):
        b, hg = core // 4, core % 4
        lo = hg * 128
        W3 = np.concatenate(
            [Wqkv[lo:lo + 128], Wqkv[C + lo:C + lo + 128],
             Wqkv[2 * C + lo:2 * C + lo + 128]])
        w3full = W3.T.reshape(4, 128, 384).transpose(1, 0, 2)  # [128,4,384]
        w3t = np.clip(w3full[:, :, 0:256] * WSCALE, -240.0, 240.0)
        w3t = np.ascontiguousarray(w3t).astype(f8)
        w3v = np.ascontiguousarray(w3full[:, :, 256:384]).astype(bf)
        b3 = np.stack([bqkv[lo:lo + 128], bqkv[C + lo:C + lo + 128]], axis=1)
        b3 = np.ascontiguousarray(b3).astype(np.float32) * np.float32(WSCALE)
        wct = np.ascontiguousarray(Wc[:, lo:lo + 128].T).astype(bf)
        in_maps.append({
            "xt": xt_b[b], "xtv": xtv_b[b], "w3t": w3t, "w3v": w3v,
            "b3": b3, "wct": wct, "tri": tri,
        })
    return in_maps


def _numpy_fallback(x, pe, content_mask, Wqkv, bqkv, Wc, bc):
    xpe = (x + pe).astype(np.float32)
    qkv = xpe.reshape(-1, C) @ Wqkv.T + bqkv
    qkv = qkv.reshape(S, B, 3 * C)
    q, k, v = np.split(qkv, 3, axis=-1)
    q = q.reshape(S, B, H, CC)
    k = k.reshape(S, B, H, CC)
    v = v.reshape(S, B, H, CC)
    out = np.empty((S, B, C), np.float32)
    for b in range(B):
        for h in range(H):
            sc = (q[:, b, h] @ k[:, b, h].T) * np.float32(TEMP)
            sc = np.where(content_mask[:, :, b], -np.inf, sc)
            sc = sc - sc.max(axis=1, keepdims=True)
            p = np.exp(sc)
            p /= p.sum(axis=1, keepdims=True)
            out[:, b, h * CC:(h + 1) * CC] = p @ v[:, b, h]
    return (out.reshape(-1, C) @ Wc.T + bc).reshape(S, B, C).astype(np.float32)


def kernel(x, pe, content_mask, pad, Wqkv, bqkv, Wc, bc):
    x = np.asarray(x, dtype=np.float32)
    pe = np.asarray(pe, dtype=np.float32)
    content_mask = np.asarray(content_mask)
    Wqkv = np.asarray(Wqkv, dtype=np.float32)
    bqkv = np.asarray(bqkv, dtype=np.float32)
    Wc = np.asarray(Wc, dtype=np.float32)
    bc = np.asarray(bc, dtype=np.float32)

    idx = np.arange(S)
    causal = idx[None, :] > idx[:, None]
    if not np.array_equal(content_mask, np.broadcast_to(causal[:, :, None], (S, S, B))):
        return _numpy_fallback(x, pe, content_mask, Wqkv, bqkv, Wc, bc)

    from concourse.bass_utils import run_bass_kernel_spmd

    nc = _get_nc()
    in_maps = _make_in_maps(x, pe, Wqkv, bqkv, Wc)
    res = run_bass_kernel_spmd(nc, in_maps, core_ids=list(range(NCORE)))
    out = np.empty((S, B, C), np.float32)
    bc_eff = bc + Wc @ bqkv[2 * C:3 * C]   # v-bias folded through the output proj
    for b in range(B):
        acc = res.results[b * 4]["outp"].astype(np.float32).copy()
        for g in range(1, 4):
            acc += res.results[b * 4 + g]["outp"]
        out[:, b, :] = acc.T + bc_eff
    return out

